# revision 20
# baseline (speedup 1.0000x reference)
"""Trainium2 Bass kernel for nn_Criterion_49237505081886.

reference semantics: the torch loop overwrites `loss` each iteration, so the
returned scalar depends ONLY on the last batch row:

    S    = sum_j (y[-1,j] - mu[-1,j])^2 / sigma[-1,j] + log(sigma[-1,j])
    loss = 0.5 * (S + NT*log(2*pi)) / (NT * BS)

The kernel ships just the last row (3 x 2048 f32 = 24 KiB) packed as
[32, 192] -- 32 DMA packets of 768 B instead of 128 x 192 B; DGE packet
processing (~16 ns/packet) dominates the input DMA's completion latency,
so fewer/fatter packets move the data-ready semaphore ~1.4 us earlier.
(Splitting across the SP+ACT queues was tried and is SLOWER: the DGE
serializes packets across queues and the second completion lands later.)

On-device pipeline:
  SP : input DMA; ACT: table preload via dummy Ln during the DMA wait
  DVE: r = recip_approx(sigma); d = y-mu; w = d*r
       custom-DVE TENSOR_TENSOR_REDUCE: m2[:,0] = C/P + sum_c(w*d)
       (fused multiply+free-dim-reduce+init; C folds in NT*log(2pi))
  ACT: m2[:,1] = sum_c Ln(sigma)  (accum_out)
  PE : ps[1,2] = svec.T @ m2  (bf16 single-pass), svec = 2^-24 = 0.5/(NT*BS)
  DVE: loss = reduce_add(ps);  SP: DMA out.

All producers bump one shared vec_sem so PE gates on a single wait.
Runs SPMD-replicated on all 8 cores; core 0's scalar is the result.
"""
import sys

if "/opt/trn_rl_repo" not in sys.path:  # harness runs from a bare directory
    sys.path.append("/opt/trn_rl_repo")

import numpy as np

LOG_2PI = 1.8378770664093453
BS, NT = 4096, 2048
P, C = 32, 64  # 2048 = 32 * 64
N_CORES = 8

SCALE = 0.5 / (NT * BS)  # == 2**-24, exact in f32 and bf16
# Per-partition accumulator seed: SCALE * P * C_COL == 0.5*NT*log(2pi)/(NT*BS)
C_COL = NT * LOG_2PI / P

_CACHE = {}


def build_nc():
    import concourse.bass as bass
    import concourse.mybir as mybir
    from concourse.dve_ops import TENSOR_TENSOR_REDUCE

    f32 = mybir.dt.float32
    bf16 = mybir.dt.bfloat16
    Act = mybir.ActivationFunctionType
    Alu = mybir.AluOpType

    nc = bass.Bass(enable_partition_id=False)
    packed_d = nc.declare_dram_parameter("packed", [P, 3 * C], f32, isOutput=False)
    loss_d = nc.declare_dram_parameter("loss", [1, 1], f32, isOutput=True)

    with (
        nc.sbuf_tensor("packed_sb", [P, 3 * C], f32) as packed_sb,
        nc.sbuf_tensor("recip", [P, C], f32) as recip,
        # f32 intermediates: bf16 was measured identical (DVE ops at [32,64]
        # are instruction-overhead-bound, ~205ns regardless of dtype).
        nc.sbuf_tensor("diff", [P, C], f32) as diff,
        nc.sbuf_tensor("w", [P, C], f32) as w,
        nc.sbuf_tensor("lnout", [P, C], f32) as lnout,
        nc.sbuf_tensor("ttr_out", [P, 1], f32) as ttr_out,
        # m2: col 0 = C/P + sum_c w*d (custom-DVE ttr accum), col 1 =
        # sum_c ln sigma (ACT accum). One matmul reduces all partitions.
        # bf16 so the fp32 LOW_HIGH double-pass matmul becomes single-pass;
        # the rounding error (~3e-4 rel) is far inside the 2e-2 gate.
        nc.sbuf_tensor("m2", [P, 2], bf16) as m2,
        nc.sbuf_tensor("svec", [P, 1], bf16) as svec,
        nc.sbuf_tensor("dum", [1, 1], f32) as dum,
        nc.sbuf_tensor("loss_sb", [1, 1], f32) as loss_sb,
        nc.psum_tensor("ps", [1, 2], f32) as ps,
        nc.semaphore("dma_sem") as dma_sem,
        nc.semaphore("vec_sem") as vec_sem,
        nc.semaphore("mm_sem") as mm_sem,
        nc.Block(no_gpsimd_drain=True) as block,
    ):
        mu_sb = packed_sb[:, 0:C]
        sg_sb = packed_sb[:, C : 2 * C]
        ty_sb = packed_sb[:, 2 * C : 3 * C]

        # Emit the input DMA inline in the main body, BEFORE the block
        # branch: SP starts the issue right at barrier release instead of
        # paying ~250ns of COMPARE_BRANCH + body-fetch dispatch first.
        nc.sync.dma_start(packed_sb[:], packed_d[:]).then_inc(dma_sem, 16)

        # Output DMA on the Pool software DGE, also inline in main (the Q7
        # pays ~600-800ns dispatch per instruction -- skipping the block
        # branch saves one). no_gpsimd_drain skips the Pool queue drain at
        # block exit, so no engine stalls on the 4-B completion -- the NEFF
        # runtime drains all DGE queues before signalling execution complete
        # (same invariant the old SP-side FINAL_DMA_WAIT=False relied on).
        nc.gpsimd.wait_ge(vec_sem, 4)
        nc.gpsimd.dma_start(
            loss_d[:], loss_sb[:], single_packet=True
        ).then_inc(dma_sem, 16)

        @block.scalar
        def _(scalar):
            # Dummy Ln on garbage (scale=0 kills the read) to pull the ACT
            # table load off the critical path, during the DMA wait.
            scalar.activation(dum[:], dum[:], Act.Ln, scale=0.0, bias=1.0)
            scalar.wait_ge(dma_sem, 16)
            with nc.allow_low_precision("bf16 lnacc, ~1e-4 rel on final loss"):
                scalar.activation(
                    lnout[:], sg_sb, Act.Ln, accum_out=m2[:, 1:2]
                ).then_inc(vec_sem, 1)

        @block.vector
        def _(vector):
            vector.memset(svec[:], SCALE).then_inc(vec_sem, 1)
            vector.wait_ge(dma_sem, 16)
            vector.reciprocal_approx_fast(recip[:], sg_sb)
            vector.tensor_sub(diff[:], ty_sb, mu_sb)
            vector.tensor_mul(w[:], diff[:], recip[:])
            # m2[:,0] = C_COL + sum_c (w*d*1.0)  -- custom-DVE fused op
            # (the ISA-native InstTensorTensorReduce wedges the DVE on this
            # compile path; the table-delivered custom op is the one that works)
            vector._custom_dve(
                TENSOR_TENSOR_REDUCE,
                out=ttr_out[:].broadcast_to([P, C]),
                in0=w[:],
                in1=diff[:],
                s0=float(C_COL),
                s1=1.0,
                accum_out=m2[:, 0:1],
            ).then_inc(vec_sem, 1)
            vector.wait_ge(mm_sem, 1)
            vector.tensor_reduce(
                loss_sb[:], ps[:], axis=mybir.AxisListType.X, op=Alu.add
            ).then_inc(vec_sem, 1)

        @block.tensor
        def _(tensor):
            # vec_sem >= 3 covers DVE's svec memset + ttr accum AND the ACT
            # lnacc column (all three producers bump the same semaphore).
            tensor.wait_ge(vec_sem, 3)
            tensor.matmul(ps[:], svec[:], m2[:], start=True, stop=True).then_inc(
                mm_sem, 1
            )

    return nc


def _get_nc():
    if "nc" not in _CACHE:
        nc = build_nc()
        # Populate .instr bytes for the ISA-encoded custom-DVE ops; raw Bass
        # skips this pass and the NEFF compiler then fails with
        # "ISA wrong length".
        from concourse.library_overlay import lower_extended_insts

        lower_extended_insts(nc)
        _CACHE["nc"] = nc
    return _CACHE["nc"]


def make_in_maps(mu, sigma, target_y):
    mu = np.asarray(mu, dtype=np.float32)
    sigma = np.asarray(sigma, dtype=np.float32)
    target_y = np.asarray(target_y, dtype=np.float32)
    packed = np.concatenate(
        [
            np.asarray(mu[-1]).reshape(P, C),
            np.asarray(sigma[-1]).reshape(P, C),
            np.asarray(target_y[-1]).reshape(P, C),
        ],
        axis=1,
    )
    packed = np.ascontiguousarray(packed)
    in_map = {"packed": packed}
    return [in_map for _ in range(N_CORES)]


def kernel(mu, sigma, target_y):
    from concourse.bass_utils import run_bass_kernel_spmd

    in_maps = make_in_maps(mu, sigma, target_y)
    res = run_bass_kernel_spmd(_get_nc(), in_maps, list(range(N_CORES))).results
    return np.asarray(res[0]["loss"], dtype=np.float32).reshape(())


# revision 24
# speedup vs baseline: 1.0002x; 1.0002x over previous
"""Trainium2 Bass kernel for nn_Criterion_49237505081886.

reference semantics: the torch loop overwrites `loss` each iteration, so the
returned scalar depends ONLY on the last batch row:

    S    = sum_j (y[-1,j] - mu[-1,j])^2 / sigma[-1,j] + log(sigma[-1,j])
    loss = 0.5 * (S + NT*log(2*pi)) / (NT * BS)

The kernel ships just the last row (3 x 2048 f32 = 24 KiB) packed as
[32, 192] -- 32 DMA packets of 768 B instead of 128 x 192 B; DGE packet
processing (~16 ns/packet) dominates the input DMA's completion latency,
so fewer/fatter packets move the data-ready semaphore ~1.4 us earlier.
(Splitting across the SP+ACT queues was tried and is SLOWER: the DGE
serializes packets across queues and the second completion lands later.)

On-device pipeline:
  SP : input DMA; ACT: table preload via dummy Ln during the DMA wait
  DVE: r = recip_approx(sigma); d = y-mu; w = d*r
       custom-DVE TENSOR_TENSOR_REDUCE: m2[:,0] = C/P + sum_c(w*d)
       (fused multiply+free-dim-reduce+init; C folds in NT*log(2pi))
  ACT: m2[:,1] = sum_c Ln(sigma)  (accum_out)
  PE : ps[1,2] = svec.T @ m2  (bf16 single-pass), svec = 2^-24 = 0.5/(NT*BS)
  DVE: loss = reduce_add(ps);  SP: DMA out.

All producers bump one shared vec_sem so PE gates on a single wait.
Runs SPMD-replicated on all 8 cores; core 0's scalar is the result.
"""
import sys

if "/opt/trn_rl_repo" not in sys.path:  # harness runs from a bare directory
    sys.path.append("/opt/trn_rl_repo")

import numpy as np

LOG_2PI = 1.8378770664093453
BS, NT = 4096, 2048
P, C = 32, 64  # 2048 = 32 * 64
N_CORES = 8

SCALE = 0.5 / (NT * BS)  # == 2**-24, exact in f32 and bf16
# Per-partition accumulator seed: SCALE * P * C_COL == 0.5*NT*log(2pi)/(NT*BS)
C_COL = NT * LOG_2PI / P

_CACHE = {}


def build_nc():
    import concourse.bass as bass
    import concourse.mybir as mybir
    from concourse.dve_ops import TENSOR_TENSOR_REDUCE

    f32 = mybir.dt.float32
    bf16 = mybir.dt.bfloat16
    Act = mybir.ActivationFunctionType
    Alu = mybir.AluOpType

    nc = bass.Bass(enable_partition_id=False)
    packed_d = nc.declare_dram_parameter("packed", [P, 3 * C], f32, isOutput=False)
    loss_d = nc.declare_dram_parameter("loss", [1, 1], f32, isOutput=True)

    with (
        nc.sbuf_tensor("packed_sb", [P, 3 * C], f32) as packed_sb,
        nc.sbuf_tensor("recip", [P, C], f32) as recip,
        # f32 intermediates: bf16 was measured identical (DVE ops at [32,64]
        # are instruction-overhead-bound, ~205ns regardless of dtype).
        nc.sbuf_tensor("diff", [P, C], f32) as diff,
        nc.sbuf_tensor("w", [P, C], f32) as w,
        nc.sbuf_tensor("lnout", [P, C], f32) as lnout,
        nc.sbuf_tensor("ttr_out", [P, 1], f32) as ttr_out,
        # m2: col 0 = C/P + sum_c w*d (custom-DVE ttr accum), col 1 =
        # sum_c ln sigma (ACT accum). One matmul reduces all partitions.
        # bf16 so the fp32 LOW_HIGH double-pass matmul becomes single-pass;
        # the rounding error (~3e-4 rel) is far inside the 2e-2 gate.
        nc.sbuf_tensor("m2", [P, 2], bf16) as m2,
        nc.sbuf_tensor("svec", [P, 1], bf16) as svec,
        nc.sbuf_tensor("dum", [1, 1], f32) as dum,
        nc.sbuf_tensor("dum2", [1, 1], f32) as dum2,
        nc.sbuf_tensor("loss_sb", [1, 1], f32) as loss_sb,
        nc.psum_tensor("ps", [1, 2], f32) as ps,
        nc.semaphore("dma_sem") as dma_sem,
        nc.semaphore("warm_sem") as warm_sem,
        nc.semaphore("vec_sem") as vec_sem,
        nc.semaphore("mm_sem") as mm_sem,
        nc.Block(no_gpsimd_drain=True) as block,
    ):
        mu_sb = packed_sb[:, 0:C]
        sg_sb = packed_sb[:, C : 2 * C]
        ty_sb = packed_sb[:, 2 * C : 3 * C]

        # Emit the input DMA inline in the main body, BEFORE the block
        # branch: SP starts the issue right at barrier release instead of
        # paying ~250ns of COMPARE_BRANCH + body-fetch dispatch first.
        nc.sync.dma_start(packed_sb[:], packed_d[:]).then_inc(dma_sem, 16)

        # Output DMA on the Pool software DGE, also inline in main (the Q7
        # pays ~600-800ns dispatch per instruction -- skipping the block
        # branch saves one). no_gpsimd_drain skips the Pool queue drain at
        # block exit, so no engine stalls on the 4-B completion -- the NEFF
        # runtime drains all DGE queues before signalling execution complete
        # (same invariant the old SP-side FINAL_DMA_WAIT=False relied on).
        # Warm-up: a throwaway 4-B Pool DMA issued while everyone waits for
        # the input -- primes the Q7's DMA-instruction decode path so the
        # real output DMA's ~810ns cold-dispatch gap shrinks. No semaphore;
        # the NEFF-exit queue drain retires it long before it matters.
        nc.gpsimd.dma_start(
            dum2[:], packed_d[0:1, 0:1], single_packet=True
        ).then_inc(warm_sem, 16)
        nc.gpsimd.wait_ge(vec_sem, 4)
        nc.gpsimd.dma_start(
            loss_d[:], loss_sb[:], single_packet=True
        ).then_inc(dma_sem, 16)

        @block.scalar
        def _(scalar):
            # Dummy Ln on garbage (scale=0 kills the read) to pull the ACT
            # table load off the critical path, during the DMA wait.
            scalar.activation(dum[:], dum[:], Act.Ln, scale=0.0, bias=1.0)
            scalar.wait_ge(dma_sem, 16)
            with nc.allow_low_precision("bf16 lnacc, ~1e-4 rel on final loss"):
                scalar.activation(
                    lnout[:], sg_sb, Act.Ln, accum_out=m2[:, 1:2]
                ).then_inc(vec_sem, 1)

        @block.vector
        def _(vector):
            vector.memset(svec[:], SCALE).then_inc(vec_sem, 1)
            vector.wait_ge(dma_sem, 16)
            vector.reciprocal_approx_fast(recip[:], sg_sb)
            vector.tensor_sub(diff[:], ty_sb, mu_sb)
            vector.tensor_mul(w[:], diff[:], recip[:])
            # m2[:,0] = C_COL + sum_c (w*d*1.0)  -- custom-DVE fused op
            # (the ISA-native InstTensorTensorReduce wedges the DVE on this
            # compile path; the table-delivered custom op is the one that works)
            vector._custom_dve(
                TENSOR_TENSOR_REDUCE,
                out=ttr_out[:].broadcast_to([P, C]),
                in0=w[:],
                in1=diff[:],
                s0=float(C_COL),
                s1=1.0,
                accum_out=m2[:, 0:1],
            ).then_inc(vec_sem, 1)
            vector.wait_ge(mm_sem, 1)
            vector.tensor_reduce(
                loss_sb[:], ps[:], axis=mybir.AxisListType.X, op=Alu.add
            ).then_inc(vec_sem, 1)

        @block.tensor
        def _(tensor):
            # vec_sem >= 3 covers DVE's svec memset + ttr accum AND the ACT
            # lnacc column (all three producers bump the same semaphore).
            tensor.wait_ge(vec_sem, 3)
            tensor.matmul(ps[:], svec[:], m2[:], start=True, stop=True).then_inc(
                mm_sem, 1
            )

    return nc


def _get_nc():
    if "nc" not in _CACHE:
        nc = build_nc()
        # Populate .instr bytes for the ISA-encoded custom-DVE ops; raw Bass
        # skips this pass and the NEFF compiler then fails with
        # "ISA wrong length".
        from concourse.library_overlay import lower_extended_insts

        lower_extended_insts(nc)
        _CACHE["nc"] = nc
    return _CACHE["nc"]


def make_in_maps(mu, sigma, target_y):
    mu = np.asarray(mu, dtype=np.float32)
    sigma = np.asarray(sigma, dtype=np.float32)
    target_y = np.asarray(target_y, dtype=np.float32)
    packed = np.concatenate(
        [
            np.asarray(mu[-1]).reshape(P, C),
            np.asarray(sigma[-1]).reshape(P, C),
            np.asarray(target_y[-1]).reshape(P, C),
        ],
        axis=1,
    )
    packed = np.ascontiguousarray(packed)
    in_map = {"packed": packed}
    return [in_map for _ in range(N_CORES)]


def kernel(mu, sigma, target_y):
    from concourse.bass_utils import run_bass_kernel_spmd

    in_maps = make_in_maps(mu, sigma, target_y)
    res = run_bass_kernel_spmd(_get_nc(), in_maps, list(range(N_CORES))).results
    return np.asarray(res[0]["loss"], dtype=np.float32).reshape(())
